# revision 2
# baseline (speedup 1.0000x reference)
"""BoundaryLoss kernel for 8 Trainium2 NeuronCores.

Computes mean_i relu(MARGIN - inputs[i, labels[i]]) over the full batch.

Strategy (data parallel over the batch dim, 8 cores):
  - Each core gets a [8192, 1024] f32 shard of `inputs` plus its labels.
  - On device: build flat element offsets row*G + label with one iota and
    one int32 add, then a single indirect DMA gathers the 8192 labeled
    elements (4 B each) straight from DRAM -- only ~32 KiB of the 32 MiB
    shard is ever read.
  - Fused relu(margin - x) + per-partition row sum on the Scalar engine
    (activation with scale=-1, bias=margin, accum_out).
  - Each core writes 128 partial sums; the host adds 1024 partials and
    divides by N.
"""

import os
import sys

for _p in ("/opt/trn_rl_repo", os.path.expanduser("~/.axon_site/_ro/trn_rl_repo")):
    if os.path.isdir(_p) and _p not in sys.path:
        sys.path.insert(0, _p)

import numpy as np

import concourse.bacc as bacc
import concourse.bass as bass
import concourse.mybir as mybir
import concourse.tile as tile
from concourse import bass_utils

POSITIVE_MARGIN = 0.99999
N, G = 65536, 1024
NCORES = 8
NS = N // NCORES  # 8192 rows per core
P = 128  # SBUF partitions


def build_program(ns: int = NS, g: int = G):
    """Build the per-core Bass program (SPMD: same program on all cores)."""
    p = P
    f = ns // p  # gathered elements per partition
    assert ns % p == 0

    nc = bacc.Bacc("TRN2", target_bir_lowering=False, debug=False)
    x_t = nc.dram_tensor("inputs", [ns, g], mybir.dt.float32, kind="ExternalInput")
    # labels arrive as int64 little-endian, reinterpreted host-side as
    # [ns, 2] int32 (low word, high word); the kernel reads the low word.
    lab_t = nc.dram_tensor("labels_lo_hi", [ns, 2], mybir.dt.int32, kind="ExternalInput")
    out_t = nc.dram_tensor("partials", [p, 1], mybir.dt.float32, kind="ExternalOutput")

    with tile.TileContext(nc) as tc:
        with tc.tile_pool(name="pool", bufs=1) as pool:
            # labels for partition q hold rows [q*f, (q+1)*f): [p, 2f] int32
            lab_sb = pool.tile([p, 2 * f], mybir.dt.int32)
            nc.sync.dma_start(
                out=lab_sb[:],
                in_=lab_t.ap().rearrange("(p f) t -> p (f t)", p=p),
            )

            # offs[q, j] = (q*f + j) * g  (flat element offset of row start)
            offs = pool.tile([p, f], mybir.dt.int32)
            nc.gpsimd.iota(
                offs[:], pattern=[[g, f]], base=0, channel_multiplier=f * g
            )
            # offs += label  (low int32 word, stride-2 view of lab_sb)
            lab_lo = lab_sb[:].rearrange("p (f t) -> p f t", t=2)[:, :, 0]
            nc.vector.tensor_tensor(
                out=offs[:], in0=offs[:], in1=lab_lo, op=mybir.AluOpType.add
            )

            # gathered[q, j] = inputs.flat[offs[q, j]]
            gath = pool.tile([p, f], mybir.dt.float32)
            nc.gpsimd.indirect_dma_start(
                out=gath[:],
                out_offset=None,
                in_=x_t.ap(),
                in_offset=bass.IndirectOffsetOnAxis(ap=offs[:], axis=1),
            )

            # relu(margin - x), summed along the free dim per partition
            margin_t = pool.tile([p, 1], mybir.dt.float32)
            nc.vector.memset(margin_t[:], POSITIVE_MARGIN)
            relu_t = pool.tile([p, f], mybir.dt.float32)
            acc = pool.tile([p, 1], mybir.dt.float32)
            nc.scalar.activation(
                out=relu_t[:],
                in_=gath[:],
                func=mybir.ActivationFunctionType.Relu,
                bias=margin_t[:],
                scale=-1.0,
                accum_out=acc[:],
            )

            nc.sync.dma_start(out=out_t.ap(), in_=acc[:])

    nc.compile()
    return nc


_PROG = None


def _get_prog():
    global _PROG
    if _PROG is None:
        _PROG = build_program()
    return _PROG


def _make_in_maps(inputs: np.ndarray, labels: np.ndarray):
    inputs = np.asarray(inputs)
    labels = np.asarray(labels)
    assert inputs.shape == (N, G), inputs.shape
    assert labels.shape == (N,), labels.shape
    inputs = np.ascontiguousarray(inputs, dtype=np.float32)

    if labels.dtype == np.int64:
        lab2 = np.ascontiguousarray(labels).view(np.int32).reshape(N, 2)
    else:
        lab2 = np.zeros((N, 2), dtype=np.int32)
        lab2[:, 0] = labels.astype(np.int32)
    lab2 = np.ascontiguousarray(lab2)

    in_maps = []
    for c in range(NCORES):
        sl = slice(c * NS, (c + 1) * NS)
        in_maps.append(
            {"inputs": inputs[sl], "labels_lo_hi": lab2[sl]}
        )
    return in_maps


def _run(inputs, labels, trace: bool = False):
    nc = _get_prog()
    in_maps = _make_in_maps(inputs, labels)
    res = bass_utils.run_bass_kernel_spmd(
        nc, in_maps, core_ids=list(range(NCORES)), trace=trace
    )
    total = 0.0
    for r in res.results:
        total += float(np.asarray(r["partials"], dtype=np.float64).sum())
    out = np.array(total / N, dtype=np.float32)
    return out, res


def kernel(inputs, labels):
    out, _ = _run(inputs, labels, trace=False)
    return out
